# revision 16
# baseline (speedup 1.0000x reference)
"""Trainium2 Bass kernel for ColaViT pre-attention QKV down-projection.

Computes gelu(hidden_states @ concat(w_q, w_k, w_v)) and splits into
(q_low, k_low, v_low), matching the fp32 jax reference.

Sharding: data-parallel on batch across 8 NeuronCores; each core owns
M=1576 token rows of the [12608, 768] x [768, 576] GEMM + exact Gelu.

v5 strategy (from v4 NTFF analysis):
- DMA_DIRECT2D trigger cost is ~0.7us FIXED per descriptor (1.5us for
  partial-partition descriptors). v4 spent ~14us of engine time on 20
  triggers; loads serialized behind a 0.73us/trigger rate on sync.
- All tiles are full 128 rows: the 40-row tail is folded into a 13th
  m-tile that overlaps the 12th by 88 rows (recompute is free - matmul
  cost depends only on the moving dim).
- w k-slices split across BOTH HWDGE rings (sync: w0/w2/w4 + x,
  scalar: w1/w3/w5) so trigger issue rate never gates w arrival; the
  x0+w wall is then pure HBM bandwidth (~1.1MB -> ~3.3us).
- Only 2 warmup matmuls (~1.2us) bridge the preamble->first-data gap;
  real stream starts ~9.3us vs ~11.2us in v4.
- x in 7 descriptors sized so each chunk lands before the PE needs it.
- Stores in 6 batches on the scalar ring; the last two are single
  m-tiles so the final store data lands ~1.2us after the last matmul.
- Tail warmup matmuls keep the PE warm into the NRT postamble.
- fp16 in/out; fp32 PSUM accumulate; exact-Gelu ACTIVATE per m-tile.
"""

import numpy as np

HIDDEN = 768
RANK = 192
N_OUT = 3 * RANK          # 576
B, S = 64, 197
N_CORES = 8
M_PER_CORE = B * S // N_CORES   # 1576
P = 128
K_TILES = HIDDEN // P     # 6
N_CHUNK = 288             # one n-half (psum bank holds 512 fp32)
N_TILES = 13              # 12 full + 1 overlapped (rows 1448..1575)
N_WARMUP_MM = 6
N_TAILWARM_MM = 8

# col offset (within the core's 1576 rows) of each m-tile
TILE_OFF = [128 * t for t in range(12)] + [M_PER_CORE - P]

# x load descriptors: list of tile-index lists
X_CHUNKS = [[0], [1], [2], [3, 4], [5, 6, 7], [8, 9, 10], [11, 12]]
# y store batches: list of tile-index lists
Y_BATCH = [[0, 1, 2], [3, 4, 5], [6, 7, 8], [9, 10], [11], [12]]

_CACHE = {}


def _build_nc():
    from contextlib import ExitStack

    import concourse.bacc as bacc
    import concourse.mybir as mybir
    from concourse.tile import TileContext

    f32 = mybir.dt.float32
    f16 = mybir.dt.float16
    gelu = mybir.ActivationFunctionType.Gelu

    nc = bacc.Bacc("TRN2", target_bir_lowering=False, debug=False,
                   num_devices=N_CORES)

    w_dram = [nc.dram_tensor(f"w{k}", [P, N_OUT], f16,
                             kind="ExternalInput") for k in range(K_TILES)]
    x_dram = [nc.dram_tensor(f"x{ci}", [P, K_TILES * P * len(ts)], f16,
                             kind="ExternalInput")
              for ci, ts in enumerate(X_CHUNKS)]
    y_dram = [nc.dram_tensor(f"y{bi}", [P, len(ts) * N_OUT], f16,
                             kind="ExternalOutput")
              for bi, ts in enumerate(Y_BATCH)]

    # map tile index -> (x chunk idx, offset within chunk)
    tile2chunk = {}
    for ci, ts in enumerate(X_CHUNKS):
        for j, t in enumerate(ts):
            tile2chunk[t] = (ci, j * P)
    tile2batch = {}
    for bi, ts in enumerate(Y_BATCH):
        for j, t in enumerate(ts):
            tile2batch[t] = (bi, j)

    with TileContext(nc) as tc, ExitStack() as ctx:
        sb = ctx.enter_context(tc.tile_pool(name="sb", bufs=1))
        pp = ctx.enter_context(tc.tile_pool(name="pp", bufs=3, space="PSUM"))

        # PE warm-up: zero tile memset on the (otherwise idle) vector
        # engine, then 2 big matmuls bridging preamble -> first data.
        zt = sb.tile([P, 520], f16, tag="zt", name="zt")
        nc.vector.memset(zt[:], 0.0)
        zps = pp.tile([8, 512], f32, tag="zps", name="zps", bufs=1)
        for _ in range(N_WARMUP_MM):
            nc.tensor.matmul(zps[:], zt[:, :8], zt[:, 8:520],
                             start=True, stop=True)

        wt = [sb.tile([P, N_OUT], f16, tag=f"w{k}", name=f"w{k}")
              for k in range(K_TILES)]
        xt = [sb.tile([P, K_TILES, P * len(ts)], f16, tag=f"x{ci}",
                      name=f"x{ci}")
              for ci, ts in enumerate(X_CHUNKS)]

        def load_x(ci):
            nc.sync.dma_start(xt[ci][:], x_dram[ci][:].rearrange(
                "p (a m) -> p a m", a=K_TILES))

        # Both rings issue their first descriptor immediately so the
        # ~2.5us first-DMA ring latency is paid once, in parallel.
        # The scalar ring pipelines descriptors ~2.2us apart (vs ~0.6us
        # on sync), so it gets exactly ONE load (w0) and no stores.
        # Sync carries everything else in consumption order; each 147KB
        # w slice then lands ~0.6us after the previous, just ahead of
        # tile-0's mid-pstate k consumption (~0.48us/slice).
        load_x(0)
        nc.scalar.dma_start(wt[0][:], w_dram[0][:])
        for k in range(1, K_TILES):
            nc.sync.dma_start(wt[k][:], w_dram[k][:])
        for ci in range(1, len(X_CHUNKS)):
            load_x(ci)

        ysb = [sb.tile([P, len(ts), N_OUT], f16, tag=f"ysb{bi}",
                       name=f"ysb{bi}")
               for bi, ts in enumerate(Y_BATCH)]

        for t in range(N_TILES):
            ci, coff = tile2chunk[t]
            bi, bj = tile2batch[t]
            ps = pp.tile([P, 2, 512], f32, tag="ps", name=f"ps{t}")
            for k in range(K_TILES):
                for nj in range(2):
                    nc.tensor.matmul(
                        ps[:, nj, :N_CHUNK],
                        xt[ci][:, k, coff:coff + P],
                        wt[k][:, nj * N_CHUNK:(nj + 1) * N_CHUNK],
                        start=(k == 0),
                        stop=(k == K_TILES - 1),
                    )
            nc.scalar.activation(ysb[bi][:, bj, :], ps[:, :, :N_CHUNK],
                                 gelu)
            if bj == len(Y_BATCH[bi]) - 1:
                nc.sync.dma_start(
                    y_dram[bi][:].rearrange("p (a n) -> p a n",
                                            a=len(Y_BATCH[bi])),
                    ysb[bi][:, :, :])

        # keep PE/NX busy into the final barrier -> warm NRT postamble
        for _ in range(N_TAILWARM_MM):
            nc.tensor.matmul(zps[:], zt[:, :8], zt[:, 8:520],
                             start=True, stop=True)

    nc.compile()
    return nc


def _get_nc():
    if "nc" not in _CACHE:
        _CACHE["nc"] = _build_nc()
    return _CACHE["nc"]


def _make_in_maps(hidden_states, w_q, w_k, w_v):
    x = np.asarray(hidden_states, dtype=np.float32).reshape(B * S, HIDDEN)
    xT16 = np.ascontiguousarray(x.T).astype(np.float16)     # [768, 12608]
    wcat = np.concatenate(
        [np.asarray(w_q, np.float32), np.asarray(w_k, np.float32),
         np.asarray(w_v, np.float32)], axis=1).astype(np.float16)

    in_maps = []
    for c in range(N_CORES):
        base = c * M_PER_CORE
        m = {f"w{k}": np.ascontiguousarray(wcat[k * P:(k + 1) * P, :])
             for k in range(K_TILES)}
        for ci, ts in enumerate(X_CHUNKS):
            segs = []
            for t in ts:
                seg = xT16[:, base + TILE_OFF[t]:base + TILE_OFF[t] + P]
                segs.append(seg.reshape(K_TILES, P, P).transpose(1, 0, 2))
            arr = np.concatenate(segs, axis=2)      # [P, K_TILES, csz]
            m[f"x{ci}"] = np.ascontiguousarray(
                arr.reshape(P, K_TILES * P * len(ts)))
        in_maps.append(m)
    return in_maps


def _postprocess(results):
    y_full = np.empty((B * S, N_OUT), dtype=np.float32)
    for c in range(N_CORES):
        base = c * M_PER_CORE
        res = results[c]
        for bi, ts in enumerate(Y_BATCH):
            buf = res[f"y{bi}"].reshape(P, len(ts), N_OUT)
            for j, t in enumerate(ts):
                off = base + TILE_OFF[t]
                y_full[off:off + P, :] = buf[:, j, :]
    y_full = y_full.reshape(B, S, N_OUT)
    q = np.ascontiguousarray(y_full[:, :, :RANK])
    k = np.ascontiguousarray(y_full[:, :, RANK:2 * RANK])
    v = np.ascontiguousarray(y_full[:, :, 2 * RANK:])
    return (q, k, v)


def kernel(hidden_states, w_q, w_k, w_v):
    from concourse.bass_utils import run_bass_kernel_spmd

    nc = _get_nc()
    in_maps = _make_in_maps(hidden_states, w_q, w_k, w_v)
    res = run_bass_kernel_spmd(nc, in_maps, list(range(N_CORES)))
    return _postprocess(res.results)
